# revision 26
# baseline (speedup 1.0000x reference)
"""MoE feed-forward (top-1 routing) Trainium2 kernel.

Strategy (v3: hidden-dim sharding)
----------------------------------
Host: gate logits + argmax replicated bit-exactly with jax on CPU (the
  reference's own op sequence), so routing always matches the oracle.
  Tokens are sorted by expert; every core sees ALL tokens.
Device (single pass, 8 cores, F-parallel): core c owns a 512-wide slice
  of the hidden dimension F for ALL experts (weights: 8.4 MB fp16 per
  core -- each expert's weights are read exactly once chip-wide).
  Per core: h[f_slice] = gelu(W1[:, f_slice]^T X^T + b1[f_slice]) for
  every token (using that token's expert weights), then the partial
  Y^T = W2[f_slice, :]^T h[f_slice] is DMA'd out in fp16.
  This is perfectly load-balanced regardless of gate skew: no padding,
  every core streams exactly T columns through the PE per layer.
Host: sum the 8 fp16 partials in fp32, add b2, scatter back to [B,L,D].
"""

import sys

if "/opt/trn_rl_repo" not in sys.path:
    sys.path.insert(0, "/opt/trn_rl_repo")

import numpy as np

import concourse.bacc as bacc
import concourse.mybir as mybir
import concourse.tile as tile

D, F, E = 1024, 4096, 4
B, L = 4, 2048
T = B * L
NC = 8
P = 128
KD = D // P          # 8  k-tiles over D
FSL = F // NC        # 512 f-slice per core
NFL = FSL // P       # 4  f-tiles per core
GROUP = 1024         # token-column group (x-load / L2-psum unit)
NG = T // GROUP      # 8 groups

TRACE = False
TRACE_CORES = None
LAST_EXEC_NS = []
LAST_TRACES = []
LAST_RESULTS = []

_cache = {}


def _run(nc, in_maps):
    import os
    import time

    from concourse import bass_utils

    trace = TRACE
    if trace:
        bass_utils.upload_artifacts = lambda d: "local://" + d

    def _go(tr):
        return bass_utils.run_bass_kernel_spmd(
            nc, in_maps, core_ids=list(range(NC)), trace=tr,
            trace_cores=TRACE_CORES,
        )

    def _go_untraced():
        # tracing infra broken (hook import failed): force-disable it for
        # the retry — run_bass_kernel_spmd also ORs in the BASS_TRACE env
        prev = os.environ.get("BASS_NEVER_TRACE")
        os.environ["BASS_NEVER_TRACE"] = "1"
        try:
            return _go(False)
        finally:
            if prev is None:
                os.environ.pop("BASS_NEVER_TRACE", None)
            else:
                os.environ["BASS_NEVER_TRACE"] = prev

    res = None
    for attempt in range(3):
        try:
            res = _go(trace)
            break
        except ModuleNotFoundError:
            trace = False
            res = _go_untraced()
            break
        except Exception as ex:
            # the device occasionally comes up wedged
            # (NRT_EXEC_UNIT_UNRECOVERABLE); retry, resetting the jax
            # backend so the retry gets a fresh PJRT client
            msg = str(ex)
            retriable = "UNRECOVERABLE" in msg or "UNAVAILABLE" in msg
            if attempt == 2 or not retriable:
                raise
            try:
                import jax
                import jax.extend.backend as _jeb

                jax.clear_caches()
                _jeb.clear_backends()
            except Exception:
                pass
            time.sleep(2.0)
    if trace:
        LAST_EXEC_NS.append(res.exec_time_ns)
        LAST_TRACES.append(
            res.instructions_and_trace[1] if res.instructions_and_trace else None
        )
        LAST_RESULTS.append(res)
    return res


def _pieces(counts):
    """Cut the expert-sorted token axis into (expert, col0, len) pieces that
    respect both the 512 grid (PSUM bank) and expert boundaries."""
    bounds = np.cumsum(counts)
    pieces = []
    pos = 0
    for e in range(E):
        end = int(bounds[e])
        while pos < end:
            nxt = min(end, (pos // 512 + 1) * 512)
            pieces.append((e, pos, nxt - pos))
            pos = nxt
    return pieces


def _build_ffn(counts):
    """counts: per-expert token counts (tuple of 4 ints summing to T)."""
    key = ("ffn3", counts)
    if key in _cache:
        return _cache[key]
    f32 = mybir.dt.float32
    f16 = mybir.dt.float16
    pieces = _pieces(counts)
    nc = bacc.Bacc("TRN2", target_bir_lowering=False, debug=False, num_devices=NC)
    # x pre-packed on host into contiguous 512-col half-group blocks:
    # block h holds tokens [h*512, (h+1)*512) as [P, KD, 512] with 8KB
    # contiguous per partition row, so one DMA trigger = 128 descriptor
    # runs (vs 1024 for a strided [D,T] slice, which backed up the DGE
    # ring for ~9us per trigger in v5).
    xt = nc.dram_tensor("xt", (2 * NG, P, KD, 512), f16, kind="ExternalInput")
    # block 0 again, as four contiguous 128-col quarter blocks: the first
    # chains are gated on 256KB + one 256KB w1 f-slice, so real compute
    # starts ~9us (right after the ~7us engine preamble) instead of ~15us
    xq = nc.dram_tensor("xq", (4, P, KD, 128), f16, kind="ExternalInput")
    # per-core f-slice weights, pre-arranged so the partition dim is first
    w1 = nc.dram_tensor("w1", (P, E, NFL, KD, P), f16, kind="ExternalInput")
    b1 = nc.dram_tensor("b1", (P, E, NFL), f32, kind="ExternalInput")
    w2 = nc.dram_tensor("w2", (P, E, KD, NFL, P), f16, kind="ExternalInput")
    yt = nc.dram_tensor("yt", (D, T), f16, kind="ExternalOutput")

    with tile.TileContext(nc) as tc:
        with (
            tc.tile_pool(name="xs", bufs=3) as xpool,
            tc.tile_pool(name="hs", bufs=1) as hpool,
            tc.tile_pool(name="wp", bufs=1) as wpool,
            tc.tile_pool(name="yp", bufs=6) as ypool,
        ):
            # One PSUM pool for everything: p1_0..3 (L1) + p2_0..3 (L2) = 8
            # banks, so no pool-transition barrier separates the layers.
            # Warm the PE clock (HAM un-throttles after ~3.4us of sustained
            # activity) with dummy matmuls into a p2 bank while the first
            # DMAs are still in flight.
            ps_cm = tc.tile_pool(name="ps", bufs=1, space="PSUM")
            psum = ps_cm.__enter__()
            # warmup operand memset runs on GpSimd (idle at t=0, no slow
            # TENSOR_LOAD init like DVE), so the dummies are gated only on
            # the Tensor engine's own ~1.2us preamble load.
            wsrc = wpool.tile([P, 512], f16, name="wsrc")
            nc.gpsimd.memset(wsrc[:], 0.0)
            wdst = psum.tile([P, 512], f32, name="p2_0")
            # bridge dummies BEFORE the first real chain: the PE queue is
            # in-order, so dummies emitted after the data-gated first matmul
            # could not cover late DMA arrival on hardware. The dummy count
            # is ALSO a clock lever: runs whose early PE activity was
            # continuous for >= ~4.5us settle at ~0.422 ns/col for the whole
            # kernel, while shorter/stuttering warmups settle at ~0.507
            # (measured: 3 or 9 dummies -> slow, >= 11 -> fast; worth ~45us
            # over this kernel). 12 cold dummies = ~5.1us of sustained ramp,
            # ending just after the quarter blocks land.
            for _ in range(12):
                nc.tensor.matmul(
                    wdst[:], wsrc[:, :P], wsrc[:], start=True, stop=True
                )
            # pre-load the Gelu table on the Act engine while DMAs stream
            gwarm = wpool.tile([P, 8], f16, name="gwarm")
            nc.scalar.activation(
                gwarm[:], wsrc[:, :8], mybir.ActivationFunctionType.Gelu
            )

            # weights: per-f-slice triggers for the first expert so the
            # first chains are gated on 256KB, not the full 1MB expert
            e_first = pieces[0][0]
            e_order = [e_first] + [e for e in range(E) if e != e_first]
            w1sb = wpool.tile([P, E, NFL, KD, P], f16, name="w1sb")
            b1sb = wpool.tile([P, E, NFL], f32, name="b1sb")
            w2sb = wpool.tile([P, E, KD, NFL, P], f16, name="w2sb")
            # ALL early-critical loads ride the sync HW DGE queue in the
            # order compute needs them (the gpsimd software DGE measured
            # only ~66 B/ns for the first w1 slices and was gating the
            # first chains until ~14us). w1 for the other experts and all
            # of w2 are NOT issued here: they are emitted on the Act queue
            # after group 0 / group 1's gelus (below), token-gated so the
            # scheduler cannot hoist them into this window.
            quarter = counts[e_first] >= 512
            nc.sync.dma_start(w1sb[:, e_first, 0], w1.ap()[:, e_first, 0])
            # b1 next: the first gelu needs it, and a late gelu blocks psum
            # bank reuse, which stalls the PE
            nc.sync.dma_start(b1sb[:], b1.ap()[:])
            if quarter:
                xqt = wpool.tile([P, 4, KD, 128], f16, name="xqt")
                for q in range(4):
                    nc.sync.dma_start(xqt[:, q], xq.ap()[q])
            for fl in range(1, NFL):
                nc.sync.dma_start(
                    w1sb[:, e_first, fl], w1.ap()[:, e_first, fl]
                )

            # x: one [P, 2, KD, 512] tile per group, loaded as two 512-col
            # half-group descriptors (contiguous 8KB per partition row).
            # Region-level deps let each 512-col piece start as soon as its
            # own half landed. Cols 0-511 instead stream via the quarter
            # blocks (their half-descriptor is skipped).
            xgs = []
            for g in range(NG):
                xg = xpool.tile([P, 2, KD, 512], f16, name="xg")
                for h in range(2):
                    if g == 0 and h == 0 and quarter:
                        continue
                    nc.sync.dma_start(xg[:, h], xt.ap()[2 * g + h])
                xgs.append(xg)

            # h[fl] spans all T columns, fp16
            hts = [hpool.tile([P, T], f16, name=f"h{fl}") for fl in range(NFL)]

            # -- layer 1 --
            l1ps = list(pieces)
            if quarter:
                # pieces[0] is exactly (e_first, 0, 512) when the first
                # expert owns >= 512 tokens; stream it as four 128-col
                # quarter pieces gated on 256KB each
                assert l1ps[0] == (e_first, 0, 512)
                l1ps[0:1] = [(e_first, q * P, P) for q in range(4)]
            by_group = [[] for _ in range(NG)]
            for pc in l1ps:
                by_group[pc[1] // GROUP].append(pc)
            for g in range(NG):
                xg = xgs[g]
                for (e, c0, ln) in by_group[g]:
                    lo = c0 - g * GROUP
                    hh, off = lo // 512, lo % 512
                    pts = [
                        psum.tile([P, 512], f32, name=f"p1_{fl}")
                        for fl in range(NFL)
                    ]
                    for fl in range(NFL):
                        for k in range(KD):
                            if quarter and g == 0 and c0 < 512:
                                rhs = xqt[:, c0 // P, k][:, 0:ln]
                            else:
                                rhs = xg[:, hh, k, off:off + ln]
                            nc.tensor.matmul(
                                pts[fl][:, :ln], w1sb[:, e, fl, k],
                                rhs,
                                start=(k == 0), stop=(k == KD - 1),
                            )
                        nc.scalar.activation(
                            hts[fl][:, c0:c0 + ln], pts[fl][:, :ln],
                            mybir.ActivationFunctionType.Gelu,
                            bias=b1sb[:, e, fl:fl + 1], scale=1.0,
                        )
                # Deferred weight loads. A plain later dma_start gets
                # HOISTED by the tile scheduler (no deps -> runs at t=0 and
                # steals early HBM bandwidth from x, measured in v5). The
                # 1-col token copy below reads a gelu output of this group
                # and writes into the weight tile, so the full-tile DMA
                # (write-after-write on the token cell, emitted later)
                # cannot be scheduled before this group's compute.
                # token = LAST gelu output column of this group (fl NFL-1),
                # so the deferred loads release only once the whole group's
                # compute is done -- an early-group token (first gelu) was
                # measured releasing 3MB+ of weight DMA at ~12us, starving
                # the critical w1[e0]/x window
                if g == 0:
                    tok = hts[NFL - 1][:, GROUP - 1:GROUP]
                    for e in e_order[1:]:
                        nc.scalar.copy(w1sb[:, e, 0, 0, 0:1], tok)
                        nc.scalar.dma_start(w1sb[:, e], w1.ap()[:, e])
                if g == 1:
                    tok = hts[NFL - 1][:, 2 * GROUP - 1:2 * GROUP]
                    for e in e_order:
                        nc.scalar.copy(w2sb[:, e, 0, 0, 0:1], tok)
                        nc.scalar.dma_start(w2sb[:, e], w2.ap()[:, e])

            # -- layer 2: partial Y^T[d] = sum_fl W2[fl,d]^T H^T[fl] --
            # one chain per (d, piece), rotating over ALL 8 psum banks.
            # Copies alternate Act (ACTIVATE-COPY ~440ns) / DVE (~830ns).
            # Only gpsimd/SP/Act can trigger DMAs; gpsimd's software DGE has
            # ~7us completion latency (it WAS the whole end-of-kernel drain
            # in v3), so it only gets stores in the first 6 of 8 d-tiles.
            l2ps = list(pieces)
            # split the very last piece so the drain ends on small transfers
            (le, lc0, lln) = l2ps[-1]
            if lln > 320:
                h1 = lln - 256
                l2ps[-1:] = [(le, lc0, h1), (le, lc0 + h1, 128),
                             (le, lc0 + h1 + 128, 128)]
            elif lln > 192:
                h1 = lln - 128
                l2ps[-1:] = [(le, lc0, h1), (le, lc0 + h1, lln - h1)]
            banks = ["p2_0", "p2_1", "p2_2", "p2_3",
                     "p1_0", "p1_1", "p1_2", "p1_3"]
            jj = 0
            for d in range(KD):
                plist = l2ps if d == KD - 1 else pieces
                for ip, (e, c0, ln) in enumerate(plist):
                    pt = psum.tile([P, 512], f32, name=banks[jj % 8])
                    for fl in range(NFL):
                        nc.tensor.matmul(
                            pt[:, :ln], w2sb[:, e, d, fl],
                            hts[fl][:, c0:c0 + ln],
                            start=(fl == 0), stop=(fl == NFL - 1),
                        )
                    ys = ypool.tile([P, 512], f16, name="ysb")
                    tail2 = d == KD - 1 and ip >= len(plist) - 3
                    if tail2:
                        # last stores pinned to the HW DGE queues
                        cp = (len(plist) - ip) % 2
                        st = [nc.sync, nc.scalar][(len(plist) - ip) % 2]
                    elif d < KD - 2:
                        cp = jj % 2
                        st = [nc.sync, nc.scalar, nc.gpsimd][jj % 3]
                    else:
                        cp = jj % 2
                        st = [nc.sync, nc.scalar][jj % 2]
                    if cp == 0:
                        nc.scalar.copy(ys[:, :ln], pt[:, :ln])
                    else:
                        nc.vector.tensor_scalar_add(
                            ys[:, :ln], pt[:, :ln], 0.0
                        )
                    st.dma_start(
                        yt.ap()[d * P:(d + 1) * P, c0:c0 + ln],
                        ys[:, :ln],
                    )
                    jj += 1
            ps_cm.__exit__(None, None, None)
    nc.compile()
    _cache[key] = nc
    return nc


def _gate_idx(x, Wg, bg):
    """Replicate the reference gate decision exactly (jax on CPU)."""
    try:
        import jax
        import jax.numpy as jnp

        with jax.default_device(jax.devices("cpu")[0]):
            gl = jnp.einsum(
                "bld,de->ble",
                jnp.asarray(x, dtype=jnp.float32),
                jnp.asarray(Wg, dtype=jnp.float32),
            ) + jnp.asarray(bg, dtype=jnp.float32)
            idx = jnp.argmax(jax.nn.softmax(gl, axis=-1), axis=-1)
        return np.asarray(idx).reshape(T)
    except Exception:
        # fallback: float64 argmax (ties at <1e-7 margin are astronomically
        # unlikely with this data distribution)
        xf = np.asarray(x, dtype=np.float64).reshape(T, D)
        gl = xf @ np.asarray(Wg, dtype=np.float64) + np.asarray(bg, np.float64)
        return np.argmax(gl, axis=1)


def kernel(x, W1, b1, W2, b2, Wg, bg):
    x = np.asarray(x, dtype=np.float32)
    W1 = np.asarray(W1, dtype=np.float32)
    b1 = np.asarray(b1, dtype=np.float32)
    W2 = np.asarray(W2, dtype=np.float32)
    b2 = np.asarray(b2, dtype=np.float32)
    Wg = np.asarray(Wg, dtype=np.float32)
    bg = np.asarray(bg, dtype=np.float32)

    xf = x.reshape(T, D)

    # ---- host routing (exact) ----
    idx = _gate_idx(x, Wg, bg)
    order = np.argsort(idx, kind="stable")
    counts = tuple(int(c) for c in np.bincount(idx, minlength=E))

    # [D, T] sorted, then packed into contiguous half-group blocks
    # [2*NG, P, KD, 512]: block h = tokens [h*512,(h+1)*512), row-major
    # (P, KD, 512) so each partition row is one contiguous 8KB run.
    XTD = xf[order].T.astype(np.float16)                      # [D, T]
    XT16 = np.ascontiguousarray(
        XTD.reshape(KD, P, 2 * NG, 512).transpose(2, 1, 0, 3)
    )                                                         # [16, P, KD, 512]
    XQ16 = np.ascontiguousarray(
        XT16[0].reshape(P, KD, 4, 128).transpose(2, 0, 1, 3)
    )                                                         # [4, P, KD, 128]

    # per-core f-slice weight tensors, partition dim first
    # W1: [E, D, F] -> [P(k in tile), E, NC, NFL, KD, P(f)]
    W1t = W1.reshape(E, KD, P, NC, NFL, P).transpose(2, 0, 3, 4, 1, 5)
    W1t = np.ascontiguousarray(W1t).astype(np.float16)  # [P, E, NC, NFL, KD, P]
    b1t = b1.reshape(E, NC, NFL, P).transpose(3, 0, 1, 2)
    b1t = np.ascontiguousarray(b1t)                      # [P, E, NC, NFL]
    # W2: [E, F, D] -> [P(f in tile), E, NC, KD, NFL, P(d)]
    W2t = W2.reshape(E, NC, NFL, P, KD, P).transpose(3, 0, 1, 4, 2, 5)
    W2t = np.ascontiguousarray(W2t).astype(np.float16)  # [P, E, NC, KD, NFL, P]

    in_maps = []
    for c in range(NC):
        in_maps.append({
            "xt": XT16,
            "xq": XQ16,
            "w1": np.ascontiguousarray(W1t[:, :, c]),
            "b1": np.ascontiguousarray(b1t[:, :, c]),
            "w2": np.ascontiguousarray(W2t[:, :, c]),
        })

    nc2 = _build_ffn(counts)
    res = _run(nc2, in_maps)

    # ---- host reduction: sum partials (fp32), add b2, unsort ----
    acc = res.results[0]["yt"].astype(np.float32)
    for c in range(1, NC):
        acc += res.results[c]["yt"].astype(np.float32)
    ys = acc.T                               # [T, D] in sorted order
    ys += b2[idx[order]]
    out = np.empty((T, D), dtype=np.float32)
    out[order] = ys
    return out.reshape(B, L, D)



# revision 32
# speedup vs baseline: 1.1908x; 1.1908x over previous
"""MoE feed-forward (top-1 routing) Trainium2 kernel.

Strategy (v3: hidden-dim sharding)
----------------------------------
Host: gate logits + argmax replicated bit-exactly with jax on CPU (the
  reference's own op sequence), so routing always matches the oracle.
  Tokens are sorted by expert; every core sees ALL tokens.
Device (single pass, 8 cores, F-parallel): core c owns a 512-wide slice
  of the hidden dimension F for ALL experts (weights: 8.4 MB fp16 per
  core -- each expert's weights are read exactly once chip-wide).
  Per core: h[f_slice] = gelu(W1[:, f_slice]^T X^T + b1[f_slice]) for
  every token (using that token's expert weights), then the partial
  Y^T = W2[f_slice, :]^T h[f_slice] is DMA'd out in fp16.
  This is perfectly load-balanced regardless of gate skew: no padding,
  every core streams exactly T columns through the PE per layer.
Host: sum the 8 fp16 partials in fp32, add b2, scatter back to [B,L,D].
"""

import sys

if "/opt/trn_rl_repo" not in sys.path:
    sys.path.insert(0, "/opt/trn_rl_repo")

import numpy as np

import concourse.bacc as bacc
import concourse.mybir as mybir
import concourse.tile as tile

D, F, E = 1024, 4096, 4
B, L = 4, 2048
T = B * L
NC = 8
P = 128
KD = D // P          # 8  k-tiles over D
FSL = F // NC        # 512 f-slice per core
NFL = FSL // P       # 4  f-tiles per core
GROUP = 1024         # token-column group (x-load / L2-psum unit)
NG = T // GROUP      # 8 groups

TRACE = False
TRACE_CORES = None
LAST_EXEC_NS = []
LAST_TRACES = []
LAST_RESULTS = []

_cache = {}


def _run(nc, in_maps):
    import os
    import time

    from concourse import bass_utils

    trace = TRACE
    if trace:
        bass_utils.upload_artifacts = lambda d: "local://" + d

    def _go(tr):
        return bass_utils.run_bass_kernel_spmd(
            nc, in_maps, core_ids=list(range(NC)), trace=tr,
            trace_cores=TRACE_CORES,
        )

    def _go_untraced():
        # tracing infra broken (hook import failed): force-disable it for
        # the retry — run_bass_kernel_spmd also ORs in the BASS_TRACE env
        prev = os.environ.get("BASS_NEVER_TRACE")
        os.environ["BASS_NEVER_TRACE"] = "1"
        try:
            return _go(False)
        finally:
            if prev is None:
                os.environ.pop("BASS_NEVER_TRACE", None)
            else:
                os.environ["BASS_NEVER_TRACE"] = prev

    res = None
    for attempt in range(3):
        try:
            res = _go(trace)
            break
        except ModuleNotFoundError:
            trace = False
            res = _go_untraced()
            break
        except Exception as ex:
            # the device occasionally comes up wedged
            # (NRT_EXEC_UNIT_UNRECOVERABLE); retry, resetting the jax
            # backend so the retry gets a fresh PJRT client
            msg = str(ex)
            retriable = "UNRECOVERABLE" in msg or "UNAVAILABLE" in msg
            if attempt == 2 or not retriable:
                raise
            try:
                import jax
                import jax.extend.backend as _jeb

                jax.clear_caches()
                _jeb.clear_backends()
            except Exception:
                pass
            time.sleep(2.0)
    if trace:
        LAST_EXEC_NS.append(res.exec_time_ns)
        LAST_TRACES.append(
            res.instructions_and_trace[1] if res.instructions_and_trace else None
        )
        LAST_RESULTS.append(res)
    return res


def _pieces(counts):
    """Cut the expert-sorted token axis into (expert, col0, len) pieces that
    respect both the 512 grid (PSUM bank) and expert boundaries."""
    bounds = np.cumsum(counts)
    pieces = []
    pos = 0
    for e in range(E):
        end = int(bounds[e])
        while pos < end:
            nxt = min(end, (pos // 512 + 1) * 512)
            pieces.append((e, pos, nxt - pos))
            pos = nxt
    return pieces


def _build_ffn(counts):
    """counts: per-expert token counts (tuple of 4 ints summing to T)."""
    key = ("ffn3", counts)
    if key in _cache:
        return _cache[key]
    f32 = mybir.dt.float32
    f16 = mybir.dt.float16
    pieces = _pieces(counts)
    nc = bacc.Bacc("TRN2", target_bir_lowering=False, debug=False, num_devices=NC)
    # x pre-packed on host into contiguous 512-col half-group blocks:
    # block h holds tokens [h*512, (h+1)*512) as [P, KD, 512] with 8KB
    # contiguous per partition row, so one DMA trigger = 128 descriptor
    # runs (vs 1024 for a strided [D,T] slice, which backed up the DGE
    # ring for ~9us per trigger in v5).
    xt = nc.dram_tensor("xt", (2 * NG, P, KD, 512), f16, kind="ExternalInput")
    # per-core f-slice weights, pre-arranged so the partition dim is first
    w1 = nc.dram_tensor("w1", (P, E, NFL, KD, P), f16, kind="ExternalInput")
    b1 = nc.dram_tensor("b1", (P, E, NFL), f32, kind="ExternalInput")
    w2 = nc.dram_tensor("w2", (P, E, KD, NFL, P), f16, kind="ExternalInput")
    yt = nc.dram_tensor("yt", (D, T), f16, kind="ExternalOutput")

    with tile.TileContext(nc) as tc:
        with (
            tc.tile_pool(name="xs", bufs=3) as xpool,
            tc.tile_pool(name="hs", bufs=1) as hpool,
            tc.tile_pool(name="wp", bufs=1) as wpool,
            tc.tile_pool(name="yp", bufs=6) as ypool,
        ):
            # One PSUM pool for everything: p1_0..3 (L1) + p2_0..3 (L2) = 8
            # banks, so no pool-transition barrier separates the layers.
            # Warm the PE clock (HAM un-throttles after ~3.4us of sustained
            # activity) with dummy matmuls into a p2 bank while the first
            # DMAs are still in flight.
            ps_cm = tc.tile_pool(name="ps", bufs=1, space="PSUM")
            psum = ps_cm.__enter__()
            # warmup operand memset runs on GpSimd (idle at t=0, no slow
            # TENSOR_LOAD init like DVE), so the dummies are gated only on
            # the Tensor engine's own ~1.2us preamble load.
            wsrc = wpool.tile([P, 512], f16, name="wsrc")
            nc.gpsimd.memset(wsrc[:], 0.0)
            wdst = psum.tile([P, 512], f32, name="p2_0")
            # bridge dummies BEFORE the first real chain: the PE queue is
            # in-order, so dummies emitted after the data-gated first matmul
            # could not cover late DMA arrival on hardware. The dummy count
            # is ALSO a clock lever: runs whose early PE activity was
            # continuous for >= ~4.5us settle at ~0.422 ns/col for the whole
            # kernel, while shorter/stuttering warmups settle at ~0.507
            # (measured: 3 or 9 dummies -> slow, >= 11 -> fast; worth ~45us
            # over this kernel). 12 cold dummies = ~5.1us of sustained ramp,
            # ending just after the quarter blocks land.
            for _ in range(12):
                nc.tensor.matmul(
                    wdst[:], wsrc[:, :P], wsrc[:], start=True, stop=True
                )
            # pre-load the Gelu table on the Act engine while DMAs stream
            gwarm = wpool.tile([P, 8], f16, name="gwarm")
            nc.scalar.activation(
                gwarm[:], wsrc[:, :8], mybir.ActivationFunctionType.Gelu
            )

            # weights: per-f-slice triggers for the first expert so the
            # first chains are gated on 256KB, not the full 1MB expert
            e_first = pieces[0][0]
            e_order = [e_first] + [e for e in range(E) if e != e_first]
            w1sb = wpool.tile([P, E, NFL, KD, P], f16, name="w1sb")
            b1sb = wpool.tile([P, E, NFL], f32, name="b1sb")
            w2sb = wpool.tile([P, E, KD, NFL, P], f16, name="w2sb")
            # ALL early-critical loads ride the sync HW DGE queue (the
            # gpsimd software DGE measured only ~66 B/ns for the first w1
            # slices and gated the first chains). Order = the deadline
            # order compute consumes them: w1[e0,fl0]+b1 and the first x
            # half-block gate the first chain (~12.5us); each later w1
            # f-slice is needed one 2.07us fl-chain later; x half-blocks
            # every ~6.9us. w1 for the other experts and all of w2 are NOT
            # issued here: they are emitted on the Act queue after group
            # 0 / group 1's gelus (below), token-gated so the scheduler
            # cannot hoist them into this window.
            # x: one [P, 2, KD, 512] tile per group, loaded as two 512-col
            # half-group descriptors (contiguous 8KB per partition row).
            # Region-level deps let each 512-col piece start as soon as its
            # own half landed.
            xgs = [xpool.tile([P, 2, KD, 512], f16, name="xg")]
            nc.sync.dma_start(w1sb[:, e_first, 0], w1.ap()[:, e_first, 0])
            nc.sync.dma_start(b1sb[:], b1.ap()[:])
            nc.sync.dma_start(xgs[0][:, 0], xt.ap()[0])
            nc.sync.dma_start(
                w1sb[:, e_first, 1], w1.ap()[:, e_first, 1]
            )
            nc.sync.dma_start(xgs[0][:, 1], xt.ap()[1])
            for fl in range(2, NFL):
                nc.sync.dma_start(
                    w1sb[:, e_first, fl], w1.ap()[:, e_first, fl]
                )
            for g in range(1, NG):
                xg = xpool.tile([P, 2, KD, 512], f16, name="xg")
                for h in range(2):
                    nc.sync.dma_start(xg[:, h], xt.ap()[2 * g + h])
                xgs.append(xg)

            # h[fl] spans all T columns, fp16
            hts = [hpool.tile([P, T], f16, name=f"h{fl}") for fl in range(NFL)]

            # -- layer 1 --
            by_group = [[] for _ in range(NG)]
            for pc in pieces:
                by_group[pc[1] // GROUP].append(pc)
            for g in range(NG):
                xg = xgs[g]
                for (e, c0, ln) in by_group[g]:
                    lo = c0 - g * GROUP
                    hh, off = lo // 512, lo % 512
                    pts = [
                        psum.tile([P, 512], f32, name=f"p1_{fl}")
                        for fl in range(NFL)
                    ]
                    for fl in range(NFL):
                        for k in range(KD):
                            nc.tensor.matmul(
                                pts[fl][:, :ln], w1sb[:, e, fl, k],
                                xg[:, hh, k, off:off + ln],
                                start=(k == 0), stop=(k == KD - 1),
                            )
                        nc.scalar.activation(
                            hts[fl][:, c0:c0 + ln], pts[fl][:, :ln],
                            mybir.ActivationFunctionType.Gelu,
                            bias=b1sb[:, e, fl:fl + 1], scale=1.0,
                        )
                # Deferred weight loads. A plain later dma_start gets
                # HOISTED by the tile scheduler (no deps -> runs at t=0 and
                # steals early HBM bandwidth from x, measured in v5). The
                # 1-col token copy below reads a gelu output of this group
                # and writes into the weight tile, so the full-tile DMA
                # (write-after-write on the token cell, emitted later)
                # cannot be scheduled before this group's compute.
                # token = LAST gelu output column of this group (fl NFL-1),
                # so the deferred loads release only once the whole group's
                # compute is done -- an early-group token (first gelu) was
                # measured releasing 3MB+ of weight DMA at ~12us, starving
                # the critical w1[e0]/x window
                if g == 0:
                    tok = hts[NFL - 1][:, GROUP - 1:GROUP]
                    for e in e_order[1:]:
                        nc.scalar.copy(w1sb[:, e, 0, 0, 0:1], tok)
                        nc.scalar.dma_start(w1sb[:, e], w1.ap()[:, e])
                if g == 1:
                    tok = hts[NFL - 1][:, 2 * GROUP - 1:2 * GROUP]
                    for e in e_order:
                        nc.scalar.copy(w2sb[:, e, 0, 0, 0:1], tok)
                        nc.scalar.dma_start(w2sb[:, e], w2.ap()[:, e])

            # -- layer 2: partial Y^T[d] = sum_fl W2[fl,d]^T H^T[fl] --
            # one chain per (d, piece), rotating over ALL 8 psum banks.
            # Copies alternate Act (ACTIVATE-COPY ~440ns) / DVE (~830ns).
            # Only gpsimd/SP/Act can trigger DMAs; gpsimd's software DGE has
            # ~7us completion latency (it WAS the whole end-of-kernel drain
            # in v3), so it only gets stores in the first 6 of 8 d-tiles.
            l2ps = list(pieces)
            # split the very last piece so the drain ends on small transfers
            (le, lc0, lln) = l2ps[-1]
            if lln > 320:
                h1 = lln - 256
                l2ps[-1:] = [(le, lc0, h1), (le, lc0 + h1, 128),
                             (le, lc0 + h1 + 128, 128)]
            elif lln > 192:
                h1 = lln - 128
                l2ps[-1:] = [(le, lc0, h1), (le, lc0 + h1, lln - h1)]
            banks = ["p2_0", "p2_1", "p2_2", "p2_3",
                     "p1_0", "p1_1", "p1_2", "p1_3"]
            jj = 0
            for d in range(KD):
                plist = l2ps if d == KD - 1 else pieces
                for ip, (e, c0, ln) in enumerate(plist):
                    pt = psum.tile([P, 512], f32, name=banks[jj % 8])
                    for fl in range(NFL):
                        nc.tensor.matmul(
                            pt[:, :ln], w2sb[:, e, d, fl],
                            hts[fl][:, c0:c0 + ln],
                            start=(fl == 0), stop=(fl == NFL - 1),
                        )
                    ys = ypool.tile([P, 512], f16, name="ysb")
                    tail2 = d == KD - 1 and ip >= len(plist) - 3
                    if tail2:
                        # last stores pinned to the HW DGE queues
                        cp = (len(plist) - ip) % 2
                        st = [nc.sync, nc.scalar][(len(plist) - ip) % 2]
                    elif d < KD - 2:
                        cp = jj % 2
                        st = [nc.sync, nc.scalar, nc.gpsimd][jj % 3]
                    else:
                        cp = jj % 2
                        st = [nc.sync, nc.scalar][jj % 2]
                    if cp == 0:
                        nc.scalar.copy(ys[:, :ln], pt[:, :ln])
                    else:
                        nc.vector.tensor_scalar_add(
                            ys[:, :ln], pt[:, :ln], 0.0
                        )
                    st.dma_start(
                        yt.ap()[d * P:(d + 1) * P, c0:c0 + ln],
                        ys[:, :ln],
                    )
                    jj += 1
            ps_cm.__exit__(None, None, None)
    nc.compile()
    _cache[key] = nc
    return nc


def _gate_idx(x, Wg, bg):
    """Replicate the reference gate decision exactly (jax on CPU)."""
    try:
        import jax
        import jax.numpy as jnp

        with jax.default_device(jax.devices("cpu")[0]):
            gl = jnp.einsum(
                "bld,de->ble",
                jnp.asarray(x, dtype=jnp.float32),
                jnp.asarray(Wg, dtype=jnp.float32),
            ) + jnp.asarray(bg, dtype=jnp.float32)
            idx = jnp.argmax(jax.nn.softmax(gl, axis=-1), axis=-1)
        return np.asarray(idx).reshape(T)
    except Exception:
        # fallback: float64 argmax (ties at <1e-7 margin are astronomically
        # unlikely with this data distribution)
        xf = np.asarray(x, dtype=np.float64).reshape(T, D)
        gl = xf @ np.asarray(Wg, dtype=np.float64) + np.asarray(bg, np.float64)
        return np.argmax(gl, axis=1)


def kernel(x, W1, b1, W2, b2, Wg, bg):
    x = np.asarray(x, dtype=np.float32)
    W1 = np.asarray(W1, dtype=np.float32)
    b1 = np.asarray(b1, dtype=np.float32)
    W2 = np.asarray(W2, dtype=np.float32)
    b2 = np.asarray(b2, dtype=np.float32)
    Wg = np.asarray(Wg, dtype=np.float32)
    bg = np.asarray(bg, dtype=np.float32)

    xf = x.reshape(T, D)

    # ---- host routing (exact) ----
    idx = _gate_idx(x, Wg, bg)
    order = np.argsort(idx, kind="stable")
    counts = tuple(int(c) for c in np.bincount(idx, minlength=E))

    # [D, T] sorted, then packed into contiguous half-group blocks
    # [2*NG, P, KD, 512]: block h = tokens [h*512,(h+1)*512), row-major
    # (P, KD, 512) so each partition row is one contiguous 8KB run.
    XTD = xf[order].T.astype(np.float16)                      # [D, T]
    XT16 = np.ascontiguousarray(
        XTD.reshape(KD, P, 2 * NG, 512).transpose(2, 1, 0, 3)
    )                                                         # [16, P, KD, 512]

    # per-core f-slice weight tensors, partition dim first
    # W1: [E, D, F] -> [P(k in tile), E, NC, NFL, KD, P(f)]
    W1t = W1.reshape(E, KD, P, NC, NFL, P).transpose(2, 0, 3, 4, 1, 5)
    W1t = np.ascontiguousarray(W1t).astype(np.float16)  # [P, E, NC, NFL, KD, P]
    b1t = b1.reshape(E, NC, NFL, P).transpose(3, 0, 1, 2)
    b1t = np.ascontiguousarray(b1t)                      # [P, E, NC, NFL]
    # W2: [E, F, D] -> [P(f in tile), E, NC, KD, NFL, P(d)]
    W2t = W2.reshape(E, NC, NFL, P, KD, P).transpose(3, 0, 1, 4, 2, 5)
    W2t = np.ascontiguousarray(W2t).astype(np.float16)  # [P, E, NC, KD, NFL, P]

    in_maps = []
    for c in range(NC):
        in_maps.append({
            "xt": XT16,
            "w1": np.ascontiguousarray(W1t[:, :, c]),
            "b1": np.ascontiguousarray(b1t[:, :, c]),
            "w2": np.ascontiguousarray(W2t[:, :, c]),
        })

    nc2 = _build_ffn(counts)
    res = _run(nc2, in_maps)

    # ---- host reduction: sum partials (fp32), add b2, unsort ----
    acc = res.results[0]["yt"].astype(np.float32)
    for c in range(1, NC):
        acc += res.results[c]["yt"].astype(np.float32)
    ys = acc.T                               # [T, D] in sorted order
    ys += b2[idx[order]]
    out = np.empty((T, D), dtype=np.float32)
    out[order] = ys
    return out.reshape(B, L, D)



# revision 33
# speedup vs baseline: 1.2159x; 1.0211x over previous
"""MoE feed-forward (top-1 routing) Trainium2 kernel.

Strategy (v3: hidden-dim sharding)
----------------------------------
Host: gate logits + argmax replicated bit-exactly with jax on CPU (the
  reference's own op sequence), so routing always matches the oracle.
  Tokens are sorted by expert; every core sees ALL tokens.
Device (single pass, 8 cores, F-parallel): core c owns a 512-wide slice
  of the hidden dimension F for ALL experts (weights: 8.4 MB fp16 per
  core -- each expert's weights are read exactly once chip-wide).
  Per core: h[f_slice] = gelu(W1[:, f_slice]^T X^T + b1[f_slice]) for
  every token (using that token's expert weights), then the partial
  Y^T = W2[f_slice, :]^T h[f_slice] is DMA'd out in fp16.
  This is perfectly load-balanced regardless of gate skew: no padding,
  every core streams exactly T columns through the PE per layer.
Host: sum the 8 fp16 partials in fp32, add b2, scatter back to [B,L,D].
"""

import sys

if "/opt/trn_rl_repo" not in sys.path:
    sys.path.insert(0, "/opt/trn_rl_repo")

import numpy as np

import concourse.bacc as bacc
import concourse.mybir as mybir
import concourse.tile as tile

D, F, E = 1024, 4096, 4
B, L = 4, 2048
T = B * L
NC = 8
P = 128
KD = D // P          # 8  k-tiles over D
FSL = F // NC        # 512 f-slice per core
NFL = FSL // P       # 4  f-tiles per core
GROUP = 1024         # token-column group (x-load / L2-psum unit)
NG = T // GROUP      # 8 groups

TRACE = False
TRACE_CORES = None
LAST_EXEC_NS = []
LAST_TRACES = []
LAST_RESULTS = []

_cache = {}


def _run(nc, in_maps):
    import os
    import time

    from concourse import bass_utils

    trace = TRACE
    if trace:
        bass_utils.upload_artifacts = lambda d: "local://" + d

    def _go(tr):
        return bass_utils.run_bass_kernel_spmd(
            nc, in_maps, core_ids=list(range(NC)), trace=tr,
            trace_cores=TRACE_CORES,
        )

    def _go_untraced():
        # tracing infra broken (hook import failed): force-disable it for
        # the retry — run_bass_kernel_spmd also ORs in the BASS_TRACE env
        prev = os.environ.get("BASS_NEVER_TRACE")
        os.environ["BASS_NEVER_TRACE"] = "1"
        try:
            return _go(False)
        finally:
            if prev is None:
                os.environ.pop("BASS_NEVER_TRACE", None)
            else:
                os.environ["BASS_NEVER_TRACE"] = prev

    res = None
    for attempt in range(3):
        try:
            res = _go(trace)
            break
        except ModuleNotFoundError:
            trace = False
            res = _go_untraced()
            break
        except Exception as ex:
            # the device occasionally comes up wedged
            # (NRT_EXEC_UNIT_UNRECOVERABLE); retry, resetting the jax
            # backend so the retry gets a fresh PJRT client
            msg = str(ex)
            retriable = "UNRECOVERABLE" in msg or "UNAVAILABLE" in msg
            if attempt == 2 or not retriable:
                raise
            try:
                import jax
                import jax.extend.backend as _jeb

                jax.clear_caches()
                _jeb.clear_backends()
            except Exception:
                pass
            time.sleep(2.0)
    if trace:
        LAST_EXEC_NS.append(res.exec_time_ns)
        LAST_TRACES.append(
            res.instructions_and_trace[1] if res.instructions_and_trace else None
        )
        LAST_RESULTS.append(res)
    return res


def _pieces(counts):
    """Cut the expert-sorted token axis into (expert, col0, len) pieces that
    respect both the 512 grid (PSUM bank) and expert boundaries."""
    bounds = np.cumsum(counts)
    pieces = []
    pos = 0
    for e in range(E):
        end = int(bounds[e])
        while pos < end:
            nxt = min(end, (pos // 512 + 1) * 512)
            pieces.append((e, pos, nxt - pos))
            pos = nxt
    return pieces


def _build_ffn(counts):
    """counts: per-expert token counts (tuple of 4 ints summing to T)."""
    key = ("ffn3", counts)
    if key in _cache:
        return _cache[key]
    f32 = mybir.dt.float32
    f16 = mybir.dt.float16
    pieces = _pieces(counts)
    nc = bacc.Bacc("TRN2", target_bir_lowering=False, debug=False, num_devices=NC)
    # x pre-packed on host into contiguous 512-col half-group blocks:
    # block h holds tokens [h*512, (h+1)*512) as [P, KD, 512] with 8KB
    # contiguous per partition row, so one DMA trigger = 128 descriptor
    # runs (vs 1024 for a strided [D,T] slice, which backed up the DGE
    # ring for ~9us per trigger in v5).
    xt = nc.dram_tensor("xt", (2 * NG, P, KD, 512), f16, kind="ExternalInput")
    # per-core f-slice weights, pre-arranged so the partition dim is first
    w1 = nc.dram_tensor("w1", (P, E, NFL, KD, P), f16, kind="ExternalInput")
    b1 = nc.dram_tensor("b1", (P, E, NFL), f32, kind="ExternalInput")
    w2 = nc.dram_tensor("w2", (P, E, KD, NFL, P), f16, kind="ExternalInput")
    yt = nc.dram_tensor("yt", (D, T), f16, kind="ExternalOutput")

    with tile.TileContext(nc) as tc:
        with (
            tc.tile_pool(name="xs", bufs=3) as xpool,
            tc.tile_pool(name="hs", bufs=1) as hpool,
            tc.tile_pool(name="wp", bufs=1) as wpool,
            tc.tile_pool(name="yp", bufs=6) as ypool,
        ):
            # One PSUM pool for everything: p1_0..3 (L1) + p2_0..3 (L2) = 8
            # banks, so no pool-transition barrier separates the layers.
            # Warm the PE clock (HAM un-throttles after ~3.4us of sustained
            # activity) with dummy matmuls into a p2 bank while the first
            # DMAs are still in flight.
            ps_cm = tc.tile_pool(name="ps", bufs=1, space="PSUM")
            psum = ps_cm.__enter__()
            # warmup operand memset runs on GpSimd (idle at t=0, no slow
            # TENSOR_LOAD init like DVE), so the dummies are gated only on
            # the Tensor engine's own ~1.2us preamble load.
            wsrc = wpool.tile([P, 512], f16, name="wsrc")
            nc.gpsimd.memset(wsrc[:], 0.0)
            wdst = psum.tile([P, 512], f32, name="p2_0")
            # bridge dummies BEFORE the first real chain: the PE queue is
            # in-order, so dummies emitted after the data-gated first matmul
            # could not cover late DMA arrival on hardware. The dummy count
            # is ALSO a clock lever: runs whose early PE activity was
            # continuous for >= ~4.5us settle at ~0.422 ns/col for the whole
            # kernel, while shorter/stuttering warmups settle at ~0.507
            # (measured: 3 or 9 dummies -> slow, >= 11 -> fast; worth ~45us
            # over this kernel). 12 cold dummies = ~5.1us of sustained ramp,
            # ending just after the quarter blocks land.
            for _ in range(12):
                nc.tensor.matmul(
                    wdst[:], wsrc[:, :P], wsrc[:], start=True, stop=True
                )
            # pre-load the Gelu table on the Act engine while DMAs stream
            gwarm = wpool.tile([P, 8], f16, name="gwarm")
            nc.scalar.activation(
                gwarm[:], wsrc[:, :8], mybir.ActivationFunctionType.Gelu
            )

            # weights: per-f-slice triggers for the first expert so the
            # first chains are gated on 256KB, not the full 1MB expert
            e_first = pieces[0][0]
            e_order = [e_first] + [e for e in range(E) if e != e_first]
            w1sb = wpool.tile([P, E, NFL, KD, P], f16, name="w1sb")
            b1sb = wpool.tile([P, E, NFL], f32, name="b1sb")
            w2sb = wpool.tile([P, E, KD, NFL, P], f16, name="w2sb")
            # ALL early-critical loads ride the sync HW DGE queue (the
            # gpsimd software DGE measured only ~66 B/ns for the first w1
            # slices and gated the first chains). Order = the deadline
            # order compute consumes them: w1[e0,fl0]+b1 and the first x
            # half-block gate the first chain (~12.5us); each later w1
            # f-slice is needed one 2.07us fl-chain later; x half-blocks
            # every ~6.9us. w1 for the other experts and all of w2 are NOT
            # issued here: they are emitted on the Act queue after group
            # 0 / group 1's gelus (below), token-gated so the scheduler
            # cannot hoist them into this window.
            # x: one [P, 2, KD, 512] tile per group, loaded as two 512-col
            # half-group descriptors (contiguous 8KB per partition row).
            # Region-level deps let each 512-col piece start as soon as its
            # own half landed.
            xgs = [xpool.tile([P, 2, KD, 512], f16, name="xg")]
            nc.sync.dma_start(w1sb[:, e_first, 0], w1.ap()[:, e_first, 0])
            nc.sync.dma_start(b1sb[:], b1.ap()[:])
            nc.sync.dma_start(xgs[0][:, 0], xt.ap()[0])
            nc.sync.dma_start(
                w1sb[:, e_first, 1], w1.ap()[:, e_first, 1]
            )
            nc.sync.dma_start(xgs[0][:, 1], xt.ap()[1])
            for fl in range(2, NFL):
                nc.sync.dma_start(
                    w1sb[:, e_first, fl], w1.ap()[:, e_first, fl]
                )
            for g in range(1, NG):
                xg = xpool.tile([P, 2, KD, 512], f16, name="xg")
                for h in range(2):
                    nc.sync.dma_start(xg[:, h], xt.ap()[2 * g + h])
                xgs.append(xg)

            # h[fl] spans all T columns, fp16
            hts = [hpool.tile([P, T], f16, name=f"h{fl}") for fl in range(NFL)]

            # -- layer 1 --
            by_group = [[] for _ in range(NG)]
            for pc in pieces:
                by_group[pc[1] // GROUP].append(pc)
            for g in range(NG):
                xg = xgs[g]
                for (e, c0, ln) in by_group[g]:
                    lo = c0 - g * GROUP
                    hh, off = lo // 512, lo % 512
                    pts = [
                        psum.tile([P, 512], f32, name=f"p1_{fl}")
                        for fl in range(NFL)
                    ]
                    for fl in range(NFL):
                        for k in range(KD):
                            nc.tensor.matmul(
                                pts[fl][:, :ln], w1sb[:, e, fl, k],
                                xg[:, hh, k, off:off + ln],
                                start=(k == 0), stop=(k == KD - 1),
                            )
                        nc.scalar.activation(
                            hts[fl][:, c0:c0 + ln], pts[fl][:, :ln],
                            mybir.ActivationFunctionType.Gelu,
                            bias=b1sb[:, e, fl:fl + 1], scale=1.0,
                        )
                # Deferred weight loads. A plain later dma_start gets
                # HOISTED by the tile scheduler (no deps -> runs at t=0 and
                # steals early HBM bandwidth from x, measured in v5). The
                # 1-col token copy below reads a gelu output of this group
                # and writes into the weight tile, so the full-tile DMA
                # (write-after-write on the token cell, emitted later)
                # cannot be scheduled before this group's compute.
                # token = LAST gelu output column of this group (fl NFL-1),
                # so the deferred loads release only once the whole group's
                # compute is done -- an early-group token (first gelu) was
                # measured releasing 3MB+ of weight DMA at ~12us, starving
                # the critical w1[e0]/x window
                # Staggered 1-2MB releases: the old 3MB@g0 + 4MB@g1 bursts
                # saturated Q10 in the 35-60us window and starved the x
                # queue (Q1 -> 0 B for 5us; the next group's first matmul
                # sat 4.7us on the x semaphore). Each w1[e] lands ~2 groups
                # before expert e's first piece; w2 is done by g4 + drain,
                # well before layer 2 (~t=110us).
                rel = {
                    0: [("w1", e_order[1])],
                    1: [("w1", e_order[2]), ("w2", e_order[0])],
                    2: [("w1", e_order[3]), ("w2", e_order[1])],
                    3: [("w2", e_order[2])],
                    4: [("w2", e_order[3])],
                }
                tok = hts[NFL - 1][:, (g + 1) * GROUP - 1:(g + 1) * GROUP]
                for kind, e in rel.get(g, []):
                    wt, wd = (w1sb, w1) if kind == "w1" else (w2sb, w2)
                    nc.scalar.copy(wt[:, e, 0, 0, 0:1], tok)
                    nc.scalar.dma_start(wt[:, e], wd.ap()[:, e])

            # -- layer 2: partial Y^T[d] = sum_fl W2[fl,d]^T H^T[fl] --
            # one chain per (d, piece), rotating over ALL 8 psum banks.
            # Copies alternate Act (ACTIVATE-COPY ~440ns) / DVE (~830ns).
            # Only gpsimd/SP/Act can trigger DMAs; gpsimd's software DGE has
            # ~7us completion latency (it WAS the whole end-of-kernel drain
            # in v3), so it only gets stores in the first 6 of 8 d-tiles.
            l2ps = list(pieces)
            # split the very last piece so the drain ends on small transfers
            (le, lc0, lln) = l2ps[-1]
            if lln > 320:
                h1 = lln - 256
                l2ps[-1:] = [(le, lc0, h1), (le, lc0 + h1, 128),
                             (le, lc0 + h1 + 128, 128)]
            elif lln > 192:
                h1 = lln - 128
                l2ps[-1:] = [(le, lc0, h1), (le, lc0 + h1, lln - h1)]
            banks = ["p2_0", "p2_1", "p2_2", "p2_3",
                     "p1_0", "p1_1", "p1_2", "p1_3"]
            jj = 0
            for d in range(KD):
                plist = l2ps if d == KD - 1 else pieces
                for ip, (e, c0, ln) in enumerate(plist):
                    pt = psum.tile([P, 512], f32, name=banks[jj % 8])
                    for fl in range(NFL):
                        nc.tensor.matmul(
                            pt[:, :ln], w2sb[:, e, d, fl],
                            hts[fl][:, c0:c0 + ln],
                            start=(fl == 0), stop=(fl == NFL - 1),
                        )
                    ys = ypool.tile([P, 512], f16, name="ysb")
                    tail2 = d == KD - 1 and ip >= len(plist) - 3
                    if tail2:
                        # last stores pinned to the HW DGE queues
                        cp = (len(plist) - ip) % 2
                        st = [nc.sync, nc.scalar][(len(plist) - ip) % 2]
                    elif d < KD - 2:
                        cp = jj % 2
                        st = [nc.sync, nc.scalar, nc.gpsimd][jj % 3]
                    else:
                        cp = jj % 2
                        st = [nc.sync, nc.scalar][jj % 2]
                    if cp == 0:
                        nc.scalar.copy(ys[:, :ln], pt[:, :ln])
                    else:
                        nc.vector.tensor_scalar_add(
                            ys[:, :ln], pt[:, :ln], 0.0
                        )
                    st.dma_start(
                        yt.ap()[d * P:(d + 1) * P, c0:c0 + ln],
                        ys[:, :ln],
                    )
                    jj += 1
            ps_cm.__exit__(None, None, None)
    nc.compile()
    _cache[key] = nc
    return nc


def _gate_idx(x, Wg, bg):
    """Replicate the reference gate decision exactly (jax on CPU)."""
    try:
        import jax
        import jax.numpy as jnp

        with jax.default_device(jax.devices("cpu")[0]):
            gl = jnp.einsum(
                "bld,de->ble",
                jnp.asarray(x, dtype=jnp.float32),
                jnp.asarray(Wg, dtype=jnp.float32),
            ) + jnp.asarray(bg, dtype=jnp.float32)
            idx = jnp.argmax(jax.nn.softmax(gl, axis=-1), axis=-1)
        return np.asarray(idx).reshape(T)
    except Exception:
        # fallback: float64 argmax (ties at <1e-7 margin are astronomically
        # unlikely with this data distribution)
        xf = np.asarray(x, dtype=np.float64).reshape(T, D)
        gl = xf @ np.asarray(Wg, dtype=np.float64) + np.asarray(bg, np.float64)
        return np.argmax(gl, axis=1)


def kernel(x, W1, b1, W2, b2, Wg, bg):
    x = np.asarray(x, dtype=np.float32)
    W1 = np.asarray(W1, dtype=np.float32)
    b1 = np.asarray(b1, dtype=np.float32)
    W2 = np.asarray(W2, dtype=np.float32)
    b2 = np.asarray(b2, dtype=np.float32)
    Wg = np.asarray(Wg, dtype=np.float32)
    bg = np.asarray(bg, dtype=np.float32)

    xf = x.reshape(T, D)

    # ---- host routing (exact) ----
    idx = _gate_idx(x, Wg, bg)
    order = np.argsort(idx, kind="stable")
    counts = tuple(int(c) for c in np.bincount(idx, minlength=E))

    # [D, T] sorted, then packed into contiguous half-group blocks
    # [2*NG, P, KD, 512]: block h = tokens [h*512,(h+1)*512), row-major
    # (P, KD, 512) so each partition row is one contiguous 8KB run.
    XTD = xf[order].T.astype(np.float16)                      # [D, T]
    XT16 = np.ascontiguousarray(
        XTD.reshape(KD, P, 2 * NG, 512).transpose(2, 1, 0, 3)
    )                                                         # [16, P, KD, 512]

    # per-core f-slice weight tensors, partition dim first
    # W1: [E, D, F] -> [P(k in tile), E, NC, NFL, KD, P(f)]
    W1t = W1.reshape(E, KD, P, NC, NFL, P).transpose(2, 0, 3, 4, 1, 5)
    W1t = np.ascontiguousarray(W1t).astype(np.float16)  # [P, E, NC, NFL, KD, P]
    b1t = b1.reshape(E, NC, NFL, P).transpose(3, 0, 1, 2)
    b1t = np.ascontiguousarray(b1t)                      # [P, E, NC, NFL]
    # W2: [E, F, D] -> [P(f in tile), E, NC, KD, NFL, P(d)]
    W2t = W2.reshape(E, NC, NFL, P, KD, P).transpose(3, 0, 1, 4, 2, 5)
    W2t = np.ascontiguousarray(W2t).astype(np.float16)  # [P, E, NC, KD, NFL, P]

    in_maps = []
    for c in range(NC):
        in_maps.append({
            "xt": XT16,
            "w1": np.ascontiguousarray(W1t[:, :, c]),
            "b1": np.ascontiguousarray(b1t[:, :, c]),
            "w2": np.ascontiguousarray(W2t[:, :, c]),
        })

    nc2 = _build_ffn(counts)
    res = _run(nc2, in_maps)

    # ---- host reduction: sum partials (fp32), add b2, unsort ----
    acc = res.results[0]["yt"].astype(np.float32)
    for c in range(1, NC):
        acc += res.results[c]["yt"].astype(np.float32)
    ys = acc.T                               # [T, D] in sorted order
    ys += b2[idx[order]]
    out = np.empty((T, D), dtype=np.float32)
    out[order] = ys
    return out.reshape(B, L, D)



# revision 34
# speedup vs baseline: 1.2311x; 1.0125x over previous
"""MoE feed-forward (top-1 routing) Trainium2 kernel.

Strategy (v3: hidden-dim sharding)
----------------------------------
Host: gate logits + argmax replicated bit-exactly with jax on CPU (the
  reference's own op sequence), so routing always matches the oracle.
  Tokens are sorted by expert; every core sees ALL tokens.
Device (single pass, 8 cores, F-parallel): core c owns a 512-wide slice
  of the hidden dimension F for ALL experts (weights: 8.4 MB fp16 per
  core -- each expert's weights are read exactly once chip-wide).
  Per core: h[f_slice] = gelu(W1[:, f_slice]^T X^T + b1[f_slice]) for
  every token (using that token's expert weights), then the partial
  Y^T = W2[f_slice, :]^T h[f_slice] is DMA'd out in fp16.
  This is perfectly load-balanced regardless of gate skew: no padding,
  every core streams exactly T columns through the PE per layer.
Host: sum the 8 fp16 partials in fp32, add b2, scatter back to [B,L,D].
"""

import sys

if "/opt/trn_rl_repo" not in sys.path:
    sys.path.insert(0, "/opt/trn_rl_repo")

import numpy as np

import concourse.bacc as bacc
import concourse.mybir as mybir
import concourse.tile as tile

D, F, E = 1024, 4096, 4
B, L = 4, 2048
T = B * L
NC = 8
P = 128
KD = D // P          # 8  k-tiles over D
FSL = F // NC        # 512 f-slice per core
NFL = FSL // P       # 4  f-tiles per core
GROUP = 1024         # token-column group (x-load / L2-psum unit)
NG = T // GROUP      # 8 groups

TRACE = False
TRACE_CORES = None
LAST_EXEC_NS = []
LAST_TRACES = []
LAST_RESULTS = []

_cache = {}


def _run(nc, in_maps):
    import os
    import time

    from concourse import bass_utils

    trace = TRACE
    if trace:
        bass_utils.upload_artifacts = lambda d: "local://" + d

    def _go(tr):
        return bass_utils.run_bass_kernel_spmd(
            nc, in_maps, core_ids=list(range(NC)), trace=tr,
            trace_cores=TRACE_CORES,
        )

    def _go_untraced():
        # tracing infra broken (hook import failed): force-disable it for
        # the retry — run_bass_kernel_spmd also ORs in the BASS_TRACE env
        prev = os.environ.get("BASS_NEVER_TRACE")
        os.environ["BASS_NEVER_TRACE"] = "1"
        try:
            return _go(False)
        finally:
            if prev is None:
                os.environ.pop("BASS_NEVER_TRACE", None)
            else:
                os.environ["BASS_NEVER_TRACE"] = prev

    res = None
    for attempt in range(3):
        try:
            res = _go(trace)
            break
        except ModuleNotFoundError:
            trace = False
            res = _go_untraced()
            break
        except Exception as ex:
            # the device occasionally comes up wedged
            # (NRT_EXEC_UNIT_UNRECOVERABLE); retry, resetting the jax
            # backend so the retry gets a fresh PJRT client
            msg = str(ex)
            retriable = "UNRECOVERABLE" in msg or "UNAVAILABLE" in msg
            if attempt == 2 or not retriable:
                raise
            try:
                import jax
                import jax.extend.backend as _jeb

                jax.clear_caches()
                _jeb.clear_backends()
            except Exception:
                pass
            time.sleep(2.0)
    if trace:
        LAST_EXEC_NS.append(res.exec_time_ns)
        LAST_TRACES.append(
            res.instructions_and_trace[1] if res.instructions_and_trace else None
        )
        LAST_RESULTS.append(res)
    return res


def _pieces(counts):
    """Cut the expert-sorted token axis into (expert, col0, len) pieces that
    respect both the 512 grid (PSUM bank) and expert boundaries."""
    bounds = np.cumsum(counts)
    pieces = []
    pos = 0
    for e in range(E):
        end = int(bounds[e])
        while pos < end:
            nxt = min(end, (pos // 512 + 1) * 512)
            pieces.append((e, pos, nxt - pos))
            pos = nxt
    return pieces


def _build_ffn(counts):
    """counts: per-expert token counts (tuple of 4 ints summing to T)."""
    key = ("ffn3", counts)
    if key in _cache:
        return _cache[key]
    f32 = mybir.dt.float32
    f16 = mybir.dt.float16
    pieces = _pieces(counts)
    nc = bacc.Bacc("TRN2", target_bir_lowering=False, debug=False, num_devices=NC)
    # x pre-packed on host into contiguous 512-col half-group blocks:
    # block h holds tokens [h*512, (h+1)*512) as [P, KD, 512] with 8KB
    # contiguous per partition row, so one DMA trigger = 128 descriptor
    # runs (vs 1024 for a strided [D,T] slice, which backed up the DGE
    # ring for ~9us per trigger in v5).
    xt = nc.dram_tensor("xt", (2 * NG, P, KD, 512), f16, kind="ExternalInput")
    # per-core f-slice weights, pre-arranged so the partition dim is first
    w1 = nc.dram_tensor("w1", (P, E, NFL, KD, P), f16, kind="ExternalInput")
    b1 = nc.dram_tensor("b1", (P, E, NFL), f32, kind="ExternalInput")
    w2 = nc.dram_tensor("w2", (P, E, KD, NFL, P), f16, kind="ExternalInput")
    yt = nc.dram_tensor("yt", (D, T), f16, kind="ExternalOutput")

    with tile.TileContext(nc) as tc:
        with (
            tc.tile_pool(name="xs", bufs=3) as xpool,
            tc.tile_pool(name="hs", bufs=1) as hpool,
            tc.tile_pool(name="wp", bufs=1) as wpool,
            tc.tile_pool(name="yp", bufs=6) as ypool,
        ):
            # One PSUM pool for everything: p1_0..3 (L1) + p2_0..3 (L2) = 8
            # banks, so no pool-transition barrier separates the layers.
            # Warm the PE clock (HAM un-throttles after ~3.4us of sustained
            # activity) with dummy matmuls into a p2 bank while the first
            # DMAs are still in flight.
            ps_cm = tc.tile_pool(name="ps", bufs=1, space="PSUM")
            psum = ps_cm.__enter__()
            # warmup operand memset runs on GpSimd (idle at t=0, no slow
            # TENSOR_LOAD init like DVE), so the dummies are gated only on
            # the Tensor engine's own ~1.2us preamble load.
            wsrc = wpool.tile([P, 512], f16, name="wsrc")
            nc.gpsimd.memset(wsrc[:], 0.0)
            wdst = psum.tile([P, 512], f32, name="p2_0")
            # bridge dummies BEFORE the first real chain: the PE queue is
            # in-order, so dummies emitted after the data-gated first matmul
            # could not cover late DMA arrival on hardware. The dummy count
            # is ALSO a clock lever: runs whose early PE activity was
            # continuous for >= ~4.5us settle at ~0.422 ns/col for the whole
            # kernel, while shorter/stuttering warmups settle at ~0.507
            # (measured: 3 or 9 dummies -> slow, >= 11 -> fast; worth ~45us
            # over this kernel). 12 cold dummies = ~5.1us of sustained ramp,
            # ending just after the quarter blocks land.
            for _ in range(12):
                nc.tensor.matmul(
                    wdst[:], wsrc[:, :P], wsrc[:], start=True, stop=True
                )
            # pre-load the Gelu table on the Act engine while DMAs stream
            gwarm = wpool.tile([P, 8], f16, name="gwarm")
            nc.scalar.activation(
                gwarm[:], wsrc[:, :8], mybir.ActivationFunctionType.Gelu
            )

            # weights: per-f-slice triggers for the first expert so the
            # first chains are gated on 256KB, not the full 1MB expert
            e_first = pieces[0][0]
            e_order = [e_first] + [e for e in range(E) if e != e_first]
            w1sb = wpool.tile([P, E, NFL, KD, P], f16, name="w1sb")
            b1sb = wpool.tile([P, E, NFL], f32, name="b1sb")
            w2sb = wpool.tile([P, E, KD, NFL, P], f16, name="w2sb")
            # ALL early-critical loads ride the sync HW DGE queue (the
            # gpsimd software DGE measured only ~66 B/ns for the first w1
            # slices and gated the first chains). Order = the deadline
            # order compute consumes them: w1[e0,fl0]+b1 and the first x
            # half-block gate the first chain (~12.5us); each later w1
            # f-slice is needed one 2.07us fl-chain later; x half-blocks
            # every ~6.9us. w1 for the other experts and all of w2 are NOT
            # issued here: they are emitted on the Act queue after group
            # 0 / group 1's gelus (below), token-gated so the scheduler
            # cannot hoist them into this window.
            # x: one [P, 2, KD, 512] tile per group, loaded as two 512-col
            # half-group descriptors (contiguous 8KB per partition row).
            # Region-level deps let each 512-col piece start as soon as its
            # own half landed.
            xgs = [xpool.tile([P, 2, KD, 512], f16, name="xg")]
            nc.sync.dma_start(w1sb[:, e_first, 0], w1.ap()[:, e_first, 0])
            nc.sync.dma_start(b1sb[:], b1.ap()[:])
            nc.sync.dma_start(xgs[0][:, 0], xt.ap()[0])
            nc.sync.dma_start(
                w1sb[:, e_first, 1], w1.ap()[:, e_first, 1]
            )
            nc.sync.dma_start(xgs[0][:, 1], xt.ap()[1])
            for fl in range(2, NFL):
                nc.sync.dma_start(
                    w1sb[:, e_first, fl], w1.ap()[:, e_first, fl]
                )
            for g in range(1, NG):
                xg = xpool.tile([P, 2, KD, 512], f16, name="xg")
                for h in range(2):
                    nc.sync.dma_start(xg[:, h], xt.ap()[2 * g + h])
                xgs.append(xg)

            # h[fl] spans all T columns, fp16
            hts = [hpool.tile([P, T], f16, name=f"h{fl}") for fl in range(NFL)]

            # -- layer 1 --
            by_group = [[] for _ in range(NG)]
            for pc in pieces:
                by_group[pc[1] // GROUP].append(pc)
            for g in range(NG):
                xg = xgs[g]
                for (e, c0, ln) in by_group[g]:
                    lo = c0 - g * GROUP
                    hh, off = lo // 512, lo % 512
                    pts = [
                        psum.tile([P, 512], f32, name=f"p1_{fl}")
                        for fl in range(NFL)
                    ]
                    for fl in range(NFL):
                        for k in range(KD):
                            nc.tensor.matmul(
                                pts[fl][:, :ln], w1sb[:, e, fl, k],
                                xg[:, hh, k, off:off + ln],
                                start=(k == 0), stop=(k == KD - 1),
                            )
                        nc.scalar.activation(
                            hts[fl][:, c0:c0 + ln], pts[fl][:, :ln],
                            mybir.ActivationFunctionType.Gelu,
                            bias=b1sb[:, e, fl:fl + 1], scale=1.0,
                        )
                # Deferred weight loads. A plain later dma_start gets
                # HOISTED by the tile scheduler (no deps -> runs at t=0 and
                # steals early HBM bandwidth from x, measured in v5). The
                # 1-col token copy below reads a gelu output of this group
                # and writes into the weight tile, so the full-tile DMA
                # (write-after-write on the token cell, emitted later)
                # cannot be scheduled before this group's compute.
                # token = LAST gelu output column of this group (fl NFL-1),
                # so the deferred loads release only once the whole group's
                # compute is done -- an early-group token (first gelu) was
                # measured releasing 3MB+ of weight DMA at ~12us, starving
                # the critical w1[e0]/x window
                # Staggered 1-2MB releases: the old 3MB@g0 + 4MB@g1 bursts
                # saturated Q10 in the 35-60us window and starved the x
                # queue (Q1 -> 0 B for 5us; the next group's first matmul
                # sat 4.7us on the x semaphore). Each w1[e] lands ~2 groups
                # before expert e's first piece; w2 is done by g4 + drain,
                # well before layer 2 (~t=110us).
                rel = {
                    0: [("w1", e_order[1])],
                    1: [("w1", e_order[2]), ("w2", e_order[0])],
                    2: [("w1", e_order[3]), ("w2", e_order[1])],
                    3: [("w2", e_order[2])],
                    4: [("w2", e_order[3])],
                }
                tok = hts[NFL - 1][:, (g + 1) * GROUP - 1:(g + 1) * GROUP]
                for kind, e in rel.get(g, []):
                    wt, wd = (w1sb, w1) if kind == "w1" else (w2sb, w2)
                    nc.scalar.copy(wt[:, e, 0, 0, 0:1], tok)
                    nc.scalar.dma_start(wt[:, e], wd.ap()[:, e])

            # -- layer 2: partial Y^T[d] = sum_fl W2[fl,d]^T H^T[fl] --
            # one chain per (d, piece), rotating over ALL 8 psum banks.
            # Copies alternate Act (ACTIVATE-COPY ~440ns) / DVE (~830ns).
            # Only gpsimd/SP/Act can trigger DMAs; gpsimd's software DGE has
            # ~7us completion latency (it WAS the whole end-of-kernel drain
            # in v3), so it only gets stores in the first 6 of 8 d-tiles.
            l2ps = list(pieces)
            # split the very last piece so the drain ends on small transfers
            (le, lc0, lln) = l2ps[-1]
            if lln > 320:
                h1 = lln - 256
                l2ps[-1:] = [(le, lc0, h1), (le, lc0 + h1, 128),
                             (le, lc0 + h1 + 128, 128)]
            elif lln > 192:
                h1 = lln - 128
                l2ps[-1:] = [(le, lc0, h1), (le, lc0 + h1, lln - h1)]
            banks = ["p2_0", "p2_1", "p2_2", "p2_3",
                     "p1_0", "p1_1", "p1_2", "p1_3"]
            jj = 0
            for d in range(KD):
                plist = l2ps if d == KD - 1 else pieces
                for ip, (e, c0, ln) in enumerate(plist):
                    pt = psum.tile([P, 512], f32, name=banks[jj % 8])
                    for fl in range(NFL):
                        nc.tensor.matmul(
                            pt[:, :ln], w2sb[:, e, d, fl],
                            hts[fl][:, c0:c0 + ln],
                            start=(fl == 0), stop=(fl == NFL - 1),
                        )
                    ys = ypool.tile([P, 512], f16, name="ysb")
                    tail2 = d == KD - 1 and ip >= len(plist) - 3
                    if tail2:
                        # last stores pinned to the HW DGE queues
                        cp = (len(plist) - ip) % 2
                        st = [nc.sync, nc.scalar][(len(plist) - ip) % 2]
                    elif d < KD - 4:
                        # gpsimd software DGE only in the first half of
                        # layer 2: its completion runs ~38us behind issue
                        # (measured Q14 packets issued at d=5 landing 7us
                        # after the last HW-queue store), so anything it
                        # gets near the end extends the kernel tail
                        cp = jj % 2
                        st = [nc.sync, nc.scalar, nc.gpsimd][jj % 3]
                    else:
                        cp = jj % 2
                        st = [nc.sync, nc.scalar][jj % 2]
                    if cp == 0:
                        nc.scalar.copy(ys[:, :ln], pt[:, :ln])
                    else:
                        nc.vector.tensor_scalar_add(
                            ys[:, :ln], pt[:, :ln], 0.0
                        )
                    st.dma_start(
                        yt.ap()[d * P:(d + 1) * P, c0:c0 + ln],
                        ys[:, :ln],
                    )
                    jj += 1
            ps_cm.__exit__(None, None, None)
    nc.compile()
    _cache[key] = nc
    return nc


def _gate_idx(x, Wg, bg):
    """Replicate the reference gate decision exactly (jax on CPU)."""
    try:
        import jax
        import jax.numpy as jnp

        with jax.default_device(jax.devices("cpu")[0]):
            gl = jnp.einsum(
                "bld,de->ble",
                jnp.asarray(x, dtype=jnp.float32),
                jnp.asarray(Wg, dtype=jnp.float32),
            ) + jnp.asarray(bg, dtype=jnp.float32)
            idx = jnp.argmax(jax.nn.softmax(gl, axis=-1), axis=-1)
        return np.asarray(idx).reshape(T)
    except Exception:
        # fallback: float64 argmax (ties at <1e-7 margin are astronomically
        # unlikely with this data distribution)
        xf = np.asarray(x, dtype=np.float64).reshape(T, D)
        gl = xf @ np.asarray(Wg, dtype=np.float64) + np.asarray(bg, np.float64)
        return np.argmax(gl, axis=1)


def kernel(x, W1, b1, W2, b2, Wg, bg):
    x = np.asarray(x, dtype=np.float32)
    W1 = np.asarray(W1, dtype=np.float32)
    b1 = np.asarray(b1, dtype=np.float32)
    W2 = np.asarray(W2, dtype=np.float32)
    b2 = np.asarray(b2, dtype=np.float32)
    Wg = np.asarray(Wg, dtype=np.float32)
    bg = np.asarray(bg, dtype=np.float32)

    xf = x.reshape(T, D)

    # ---- host routing (exact) ----
    idx = _gate_idx(x, Wg, bg)
    order = np.argsort(idx, kind="stable")
    counts = tuple(int(c) for c in np.bincount(idx, minlength=E))

    # [D, T] sorted, then packed into contiguous half-group blocks
    # [2*NG, P, KD, 512]: block h = tokens [h*512,(h+1)*512), row-major
    # (P, KD, 512) so each partition row is one contiguous 8KB run.
    XTD = xf[order].T.astype(np.float16)                      # [D, T]
    XT16 = np.ascontiguousarray(
        XTD.reshape(KD, P, 2 * NG, 512).transpose(2, 1, 0, 3)
    )                                                         # [16, P, KD, 512]

    # per-core f-slice weight tensors, partition dim first
    # W1: [E, D, F] -> [P(k in tile), E, NC, NFL, KD, P(f)]
    W1t = W1.reshape(E, KD, P, NC, NFL, P).transpose(2, 0, 3, 4, 1, 5)
    W1t = np.ascontiguousarray(W1t).astype(np.float16)  # [P, E, NC, NFL, KD, P]
    b1t = b1.reshape(E, NC, NFL, P).transpose(3, 0, 1, 2)
    b1t = np.ascontiguousarray(b1t)                      # [P, E, NC, NFL]
    # W2: [E, F, D] -> [P(f in tile), E, NC, KD, NFL, P(d)]
    W2t = W2.reshape(E, NC, NFL, P, KD, P).transpose(3, 0, 1, 4, 2, 5)
    W2t = np.ascontiguousarray(W2t).astype(np.float16)  # [P, E, NC, KD, NFL, P]

    in_maps = []
    for c in range(NC):
        in_maps.append({
            "xt": XT16,
            "w1": np.ascontiguousarray(W1t[:, :, c]),
            "b1": np.ascontiguousarray(b1t[:, :, c]),
            "w2": np.ascontiguousarray(W2t[:, :, c]),
        })

    nc2 = _build_ffn(counts)
    res = _run(nc2, in_maps)

    # ---- host reduction: sum partials (fp32), add b2, unsort ----
    acc = res.results[0]["yt"].astype(np.float32)
    for c in range(1, NC):
        acc += res.results[c]["yt"].astype(np.float32)
    ys = acc.T                               # [T, D] in sorted order
    ys += b2[idx[order]]
    out = np.empty((T, D), dtype=np.float32)
    out[order] = ys
    return out.reshape(B, L, D)

